# revision 15
# baseline (speedup 1.0000x reference)
"""AttentionHead kernel for Trainium2, 8 NeuronCores, data-parallel over batch.

Problem (fixed shapes):
    input_tensor [8, 2048, 1024] f32, attention_mask [8, 2048] int64 (0/1),
    Wq/Wk/Wv [1024, 128] f32, bq/bk/bv [128] f32.
    out = softmax(mask(Q @ K^T / sqrt(2048))) @ V    -> [8, 2048, 128] f32

Sharding: one batch element per core (B == n_cores == 8). No collectives.

Per-core device kernel (bf16 inputs, f32 accumulation), v3:
  - Host pre-transposes X -> XT [1024, 2048] bf16, prepacks W into the SBUF
    layout [128, 8*128] (contiguous 2KB partition lines -> fast DMA), folds
    1/sqrt(S) into Wq/bq.
  - DMA issue is split across the two HWDGE queues (sync + scalar) so issue
    overhead (~0.6us per DMA instruction) overlaps; a dummy exp at t=0
    preloads the ACT exp table during the X DMA.
  - Pass 1 (DMA-paced, per chunk): KT all 4 blocks + QT blocks 0,1.
    Bias is folded into the PSUM->SBUF cast (per-partition tensor_scalar).
  - Score/exp tiles cover ONE key chunk x TWO query blocks [128, 1024]:
    within a tile the mask depends only on the partition (the key), so the
    attention mask folds into the exp's per-partition bias (0 or -50) and
    E comes out of ScalarE already masked. 32 activations total; no
    max-subtraction (|scores| <= ~1.5 by construction).
  - Softmax denominator: plain running TENSOR_TENSOR adds over the 16
    pre-masked E tiles per block pair, one [K=128,M=1,N=512] matmul with a
    ones column, reciprocal_approx_fast (DVE), gpsimd partition_broadcast,
    final DVE multiply. The chain is emitted early so it overlaps AV.
  - Emission interleaves: scores(blocks 0,1) into the remaining projections;
    scores(blocks 2,3) + AV(0..3) in one loop, AV of blocks 2,3 lagging two
    key chunks so every engine stays busy. PE never idles > ~1us, so the
    HAM clock gate stays open.
  - Output written as OT [128, 2048]; host transposes to [2048, 128].
"""

import sys

for _p in ("/opt/trn_rl_repo", "/root/.axon_site/_ro/trn_rl_repo"):
    if _p not in sys.path:
        sys.path.append(_p)

import numpy as np
import ml_dtypes

B, S, DIN, DOUT = 8, 2048, 1024, 128
NCHUNK = DIN // 128          # 8 contraction chunks
NKEY = S // 128              # 16 key chunks
QBLK = 512                   # query block (free dim of S^T / OT matmuls)
NQB = S // QBLK              # 4 query blocks

BF16 = ml_dtypes.bfloat16


def _build():
    import concourse.bass as bass
    import concourse.tile as tile
    from concourse import bacc, mybir
    from concourse.masks import make_identity

    f32 = mybir.dt.float32
    bf16 = mybir.dt.bfloat16
    Exp = mybir.ActivationFunctionType.Exp

    nc = bacc.Bacc("TRN2", target_bir_lowering=False, debug=False, num_devices=B)

    xt_d = nc.dram_tensor("xt", [DIN, S], bf16, kind="ExternalInput")
    wq_d = nc.dram_tensor("wq", [128, DIN], bf16, kind="ExternalInput")
    wk_d = nc.dram_tensor("wk", [128, DIN], bf16, kind="ExternalInput")
    wv_d = nc.dram_tensor("wv", [128, DIN], bf16, kind="ExternalInput")
    bcol_d = nc.dram_tensor("bcol", [128, 4], f32, kind="ExternalInput")
    mb_d = nc.dram_tensor("mb", [128, NKEY], f32, kind="ExternalInput")
    out_d = nc.dram_tensor("out", [DOUT, S], f32, kind="ExternalOutput")

    with tile.TileContext(nc) as tc:
        with (
            tc.tile_pool(name="persist", bufs=1) as pp,
            tc.tile_pool(name="epool", bufs=20) as ep,
            tc.tile_pool(name="tree", bufs=6) as tp,
            tc.tile_pool(name="normp", bufs=2) as rp,
            tc.tile_pool(name="outp", bufs=2) as op,
        ):
            xts = [pp.tile([128, S], bf16, tag=f"xt{c}", name=f"xt{c}")
                   for c in range(NCHUNK)]
            wq = pp.tile([128, DIN], bf16, tag="wq")
            wk = pp.tile([128, DIN], bf16, tag="wk")
            wv = pp.tile([128, DIN], bf16, tag="wv")
            bcol = pp.tile([128, 4], f32, tag="bcol")
            mb = pp.tile([128, NKEY], f32, tag="mb")
            ocol = pp.tile([128, 1], bf16, tag="ocol")
            ident = pp.tile([128, 128], bf16, tag="ident")
            qt = pp.tile([128, S], bf16, tag="qt")
            kt = pp.tile([128, S], bf16, tag="kt")
            vt = pp.tile([128, S], bf16, tag="vt")
            vn = pp.tile([128, S], bf16, tag="vn")
            wrm_i = pp.tile([1, 32], f32, tag="wrm_i")
            wrm_o = pp.tile([1, 32], f32, tag="wrm_o")

            # exp table preload (overlaps the input DMA)
            nc.vector.memset(wrm_i[:], 0.0)
            nc.scalar.activation(wrm_o[:], wrm_i[:], Exp)

            nc.vector.memset(ocol[:], 1.0)
            make_identity(nc, ident[:])

            # PE warm-up scratch: ~4us of dummy matmuls during the X DMA
            # wait flips the HAM clock gate to 8/8 before pass 1 starts.
            wrm_l = pp.tile([128, 128], bf16, tag="wrm_l")
            wrm_r = pp.tile([128, QBLK], bf16, tag="wrm_r")
            nc.vector.memset(wrm_l[:], 0.0)
            nc.vector.memset(wrm_r[:], 0.0)

            # DMAs. All X chunks go on the sync queue in consumption order:
            # serial issue (~0.7us each) keeps only ~2 transfers in flight,
            # so chunks complete near-sequentially (packet round-robin would
            # otherwise delay chunk 0 to the end). Small tensors ride the
            # scalar queue.
            xt3 = xt_d.ap().rearrange("(c p) m -> p c m", p=128)
            for c in range(NCHUNK):
                nc.sync.dma_start(xts[c][:, 0:1024], xt3[:, c, 0:1024])
                nc.sync.dma_start(xts[c][:, 1024:2048], xt3[:, c, 1024:2048])
            nc.scalar.dma_start(wk[:], wk_d.ap())
            nc.scalar.dma_start(wq[:], wq_d.ap())
            nc.scalar.dma_start(mb[:], mb_d.ap())
            nc.scalar.dma_start(bcol[:], bcol_d.ap())
            nc.scalar.dma_start(wv[:], wv_d.ap())

            def cast_bias(dst_sl, src, col):
                nc.vector.tensor_scalar_add(dst_sl, src, bcol[:, col:col + 1])

            egs = {}      # (pair, j) -> E tile [128, 1024] bf16, pre-masked
            rlast = {}    # pair -> latest running-sum tile [128, 1024] bf16
            pds = {}      # t -> denominator PSUM tile [1, 512]
            rdbs = {}     # t -> broadcast reciprocal [128, 512] f32
            pots = {}     # t -> AV accumulator PSUM tile

            # ---------- pass 1: KT (all blocks) + QT blocks 0,1 ----------
            with tc.tile_pool(name="psA1", bufs=1, space="PSUM") as psA1:
                psK = [psA1.tile([128, QBLK], f32, tag=f"pk{t}", name=f"pk{t}")
                       for t in range(NQB)]
                psQ0 = psA1.tile([128, QBLK], f32, tag="pq0")
                psQ1 = psA1.tile([128, QBLK], f32, tag="pq1")
                psW = psA1.tile([128, QBLK], f32, tag="pwrm")
                for _ in range(10):
                    nc.tensor.matmul(psW[:], wrm_l[:], wrm_r[:],
                                     start=True, stop=True)
                for c in range(NCHUNK):
                    wks = wk[:, c * 128:(c + 1) * 128]
                    wqs = wq[:, c * 128:(c + 1) * 128]
                    st, sp = (c == 0), (c == NCHUNK - 1)
                    nc.tensor.matmul(psK[0][:], wks, xts[c][:, 0:512],
                                     start=st, stop=sp)
                    nc.tensor.matmul(psK[1][:], wks, xts[c][:, 512:1024],
                                     start=st, stop=sp)
                    nc.tensor.matmul(psQ0[:], wqs, xts[c][:, 0:512],
                                     start=st, stop=sp)
                    nc.tensor.matmul(psQ1[:], wqs, xts[c][:, 512:1024],
                                     start=st, stop=sp)
                    nc.tensor.matmul(psK[2][:], wks, xts[c][:, 1024:1536],
                                     start=st, stop=sp)
                    nc.tensor.matmul(psK[3][:], wks, xts[c][:, 1536:2048],
                                     start=st, stop=sp)
                # casts split across ScalarE (idle until the first exp) and
                # DVE, with s(0, j=0..3) dependencies (K block 0, Q0, Q1)
                # first so the exp stream starts ~3us earlier.
                Ident = mybir.ActivationFunctionType.Identity
                nc.scalar.activation(kt[:, 0:QBLK], psK[0][:], Ident,
                                     bias=bcol[:, 1:2])
                cast_bias(qt[:, 0:QBLK], psQ0[:], 0)
                nc.scalar.activation(kt[:, QBLK:2 * QBLK], psK[1][:], Ident,
                                     bias=bcol[:, 1:2])
                cast_bias(qt[:, QBLK:2 * QBLK], psQ1[:], 0)
                nc.scalar.activation(kt[:, 2 * QBLK:3 * QBLK], psK[2][:],
                                     Ident, bias=bcol[:, 1:2])
                nc.scalar.activation(kt[:, 3 * QBLK:4 * QBLK], psK[3][:],
                                     Ident, bias=bcol[:, 1:2])

            # ---------- phase C pools (ps_st lives through the R region) ----
            with tc.tile_pool(name="ps_st", bufs=2, space="PSUM") as ps_st:

                def emit_score_pair(pair, j):
                    """pair 0 -> q blocks 0,1; pair 1 -> q blocks 2,3."""
                    pst = ps_st.tile([128, 2 * QBLK], f32, tag="st",
                                     name=f"pst{pair}_{j}")
                    ktj = kt[:, j * 128:(j + 1) * 128]
                    for half in (0, 1):
                        t = 2 * pair + half
                        nc.tensor.matmul(
                            pst[:, half * QBLK:(half + 1) * QBLK],
                            ktj, qt[:, t * QBLK:(t + 1) * QBLK],
                            start=True, stop=True,
                        )
                    eg = ep.tile([128, 2 * QBLK], bf16, tag="e",
                                 name=f"eg{pair}_{j}")
                    nc.scalar.activation(eg[:], pst[:], Exp,
                                         bias=mb[:, j:j + 1])
                    egs[(pair, j)] = eg

                def emit_tree(pair, j):
                    """running masked-E sum for a block pair (plain adds)."""
                    eg = egs[(pair, j)]
                    if j == 0:
                        rlast[pair] = eg
                        return
                    r = tp.tile([128, 2 * QBLK], bf16, tag="r",
                                name=f"r{pair}_{j}")
                    nc.vector.tensor_add(r[:], rlast[pair][:], eg[:])
                    rlast[pair] = r

                def emit_av(t, j):
                    if j == 0:
                        pots[t] = ps_o.tile([128, QBLK], f32, tag="o",
                                            name=f"pot{t}")
                    eg = egs[(t // 2, j)]
                    half = t % 2
                    nc.tensor.matmul(
                        pots[t][:],
                        vn[:, j * 128:(j + 1) * 128],
                        eg[:, half * QBLK:(half + 1) * QBLK],
                        start=(j == 0), stop=(j == NKEY - 1),
                    )

                def emit_pd(t):
                    """denominator matmul + reciprocal + broadcast (early)."""
                    half = t % 2
                    pd = ps_m.tile([1, QBLK], f32, tag="d", name=f"pd{t}")
                    nc.tensor.matmul(
                        pd[:], ocol[:],
                        rlast[t // 2][:, half * QBLK:(half + 1) * QBLK],
                        start=True, stop=True)
                    rdc = rp.tile([1, QBLK], f32, tag="rdc", name=f"rdc{t}")
                    nc.vector.reciprocal_approx_fast(rdc[:], pd[:])
                    rdb = rp.tile([128, QBLK], f32, tag="rdb", name=f"rdb{t}")
                    nc.gpsimd.partition_broadcast(rdb[:], rdc[:])
                    rdbs[t] = rdb

                def emit_finish(t):
                    osb = op.tile([128, QBLK], f32, tag="osb", name=f"osb{t}")
                    nc.vector.tensor_mul(osb[:], pots[t][:], rdbs[t][:])
                    nc.sync.dma_start(
                        out_d.ap()[:, t * QBLK:(t + 1) * QBLK], osb[:])

                # ---- R region: rest of projections + V transposes,
                # interleaved with score pairs for blocks 0,1 ----
                with (
                    tc.tile_pool(name="psA2", bufs=2, space="PSUM") as psA2,
                    tc.tile_pool(name="ps_tr", bufs=2, space="PSUM") as ps_tr,
                ):
                    r_ops = []  # PE-op thunks, 4 consumed per score pair

                    def proj8(w, t, dst, col, nm):
                        pr = psA2.tile([128, QBLK], f32, tag="pr",
                                       name=f"pr_{nm}")
                        for c in range(NCHUNK):
                            r_ops.append(lambda c=c, pr=pr: nc.tensor.matmul(
                                pr[:],
                                w[:, c * 128:(c + 1) * 128],
                                xts[c][:, t * QBLK:(t + 1) * QBLK],
                                start=(c == 0), stop=(c == NCHUNK - 1)))
                        r_ops.append(lambda pr=pr: cast_bias(
                            dst[:, t * QBLK:(t + 1) * QBLK], pr[:], col))

                    def tr1(k):
                        ptr = ps_tr.tile([128, 128], bf16, tag="tr",
                                         name=f"tr{k}")
                        nc.tensor.transpose(
                            ptr[:], vt[:, k * 128:(k + 1) * 128], ident[:])
                        nc.vector.tensor_copy(
                            vn[:, k * 128:(k + 1) * 128], ptr[:])

                    proj8(wq, 2, qt, 0, "q2")
                    proj8(wq, 3, qt, 0, "q3")
                    for t in range(NQB):
                        proj8(wv, t, vt, 2, f"v{t}")
                        for k in range(4 * t, 4 * t + 4):
                            r_ops.append(lambda k=k: tr1(k))

                    ri = 0
                    for j in range(NKEY):
                        emit_score_pair(0, j)
                        emit_tree(0, j)
                        for _ in range(4):
                            if ri < len(r_ops):
                                r_ops[ri]()
                                ri += 1
                    while ri < len(r_ops):
                        r_ops[ri]()
                        ri += 1

                # ---- steady phase C ----
                # PSUM budget: ps_st 8KB + 3 concurrent pots 6KB + pd 2KB
                # = 16KB exactly, so AV for block 3 trails the loop.
                with (
                    tc.tile_pool(name="ps_o", bufs=3, space="PSUM") as ps_o,
                    tc.tile_pool(name="ps_m", bufs=1, space="PSUM") as ps_m,
                ):
                    # first half: drain ALL of blocks 0,1 AV (pair-0 E tiles
                    # all exist already) while scores/exp for pair 1 stream.
                    for j in range(NKEY // 2):
                        emit_score_pair(1, j)
                        emit_av(0, 2 * j)
                        emit_av(0, 2 * j + 1)
                        emit_av(1, 2 * j)
                        emit_av(1, 2 * j + 1)
                        if j == 1:
                            emit_pd(0)
                        if j == 2:
                            emit_pd(1)
                        emit_tree(1, j)
                    emit_finish(0)
                    emit_finish(1)
                    # second half: blocks 2,3 AV lag the pair-1 exp stream.
                    for j in range(NKEY // 2, NKEY):
                        emit_score_pair(1, j)
                        k = 2 * (j - NKEY // 2)
                        emit_av(2, k)
                        emit_av(2, k + 1)
                        emit_av(3, k)
                        emit_av(3, k + 1)
                        emit_tree(1, j)
                    emit_pd(2)
                    emit_pd(3)
                    emit_finish(2)
                    emit_finish(3)

    nc.compile()
    return nc


_NC = None


def _get_nc():
    global _NC
    if _NC is None:
        _NC = _build()
    return _NC


def _prep_in_maps(input_tensor, attention_mask, Wq, bq, Wk, bk, Wv, bv):
    scale = np.float32(1.0 / np.sqrt(np.float32(S)))

    def pack_w(w, sc=None):
        w = np.asarray(w, np.float32)
        if sc is not None:
            w = w * sc
        # [1024, 128] -> [128, 8*128]: row c*128+p, col e -> [p, c*128+e]
        return np.ascontiguousarray(
            w.reshape(NCHUNK, 128, DOUT).transpose(1, 0, 2).reshape(128, DIN)
        ).astype(BF16)

    wq_h = pack_w(Wq, scale)
    wk_h = pack_w(Wk)
    wv_h = pack_w(Wv)
    bcol_h = np.zeros((128, 4), np.float32)
    bcol_h[:, 0] = np.asarray(bq, np.float32) * scale
    bcol_h[:, 1] = np.asarray(bk, np.float32)
    bcol_h[:, 2] = np.asarray(bv, np.float32)

    x = np.asarray(input_tensor, np.float32)
    m = np.asarray(attention_mask)
    in_maps = []
    for b in range(B):
        xt_h = np.ascontiguousarray(x[b].T).astype(BF16)            # [DIN, S]
        # exp bias per (key % 128, key chunk): 0 keep, -50 mask
        mb_h = np.ascontiguousarray(
            (m[b].astype(np.float32).reshape(NKEY, 128).T - 1.0) * 50.0)
        in_maps.append({
            "xt": xt_h, "wq": wq_h, "wk": wk_h, "wv": wv_h,
            "bcol": bcol_h, "mb": mb_h,
        })
    return in_maps


def run(in_maps, trace=False, **kwargs):
    from concourse.bass_utils import run_bass_kernel_spmd

    nc = _get_nc()
    return run_bass_kernel_spmd(
        nc, in_maps, core_ids=list(range(B)), trace=trace, **kwargs
    )


def kernel(input_tensor, attention_mask, Wq, bq, Wk, bk, Wv, bv):
    in_maps = _prep_in_maps(
        input_tensor, attention_mask, Wq, bq, Wk, bk, Wv, bv)
    res = run(in_maps, trace=False)
    out = np.stack([res.results[b]["out"].T for b in range(B)])
    return np.ascontiguousarray(out.astype(np.float32))
